# revision 21
# baseline (speedup 1.0000x reference)
"""Trainium2 Bass kernel for CollapsedPBFA (collapsed Chebyshev linear attention).

Full-input contract: kernel(x, W_in, W_out) -> (B, S, D) float32.

Sharding: 8 cores = (batch b in {0,1}) x (head-block hb in {0..3} of 4 heads).
QKV is column-parallel per head block; the output projection is row-parallel
(each core computes a partial (S, D) product over its 256 hidden columns) and
the host sums per-core partials per batch element.

v2 layout/engine plan (all bf16 operands, fp32 PSUM):
  - beta is nonzero only for Chebyshev orders 1..5; beta_p is folded into the
    per-p lower-triangular cumsum constants (features) and applied to the
    den channels via a tiny broadcast multiply (den goes through an unscaled
    triangle).
  - kv PSUM layout per s-tile: [p0|p1|p2|p3|p4|den20] at 256-col offsets
    (1300 f32 = 3 banks); carry is 3 banked rank-1 matmuls (512/512/276).
  - Chebyshev chain splits work: Act takes single-input affine ops
    (copies, u-1), DVE takes the tensor-tensor chain at 4-s-tile granularity,
    Pool takes medium copies.  kvt evacuation is split DVE/Act/Pool per site
    to shorten the serial tri->evac->tri carry chain.
  - PE emission is software-pipelined (QKV / cumsum / transpose+proj
    interleaved) to avoid gaps (PE p-state ramp: gaps halve the clock).
  - Output PART is bf16 (host upcasts and sums), halving output DMA.
"""

import sys

for _p in ("/opt/trn_rl_repo", "/root/.axon_site/_ro/trn_rl_repo"):
    if _p not in sys.path:
        sys.path.append(_p)

import os

import numpy as np

import concourse.bacc as bacc
import concourse.bass as bass
import concourse.tile as tile
from concourse import mybir

if os.environ.get("LDWOPT", "0") == "1":
    import concourse.bass_utils as _bu

    if not getattr(_bu, "_ldwopt_patched", False):
        _orig_run_command = _bu.run_command

        def _run_command_ldwopt(cmd, *a, **kw):
            cmd = ["--enable-ldw-opt=true" if c == "--enable-ldw-opt=false" else c
                   for c in cmd]
            return _orig_run_command(cmd, *a, **kw)

        _bu.run_command = _run_command_ldwopt
        _bu._ldwopt_patched = True

F32 = mybir.dt.float32
BF16 = mybir.dt.bfloat16

B, S, D = 2, 1024, 1024
H, DH = 16, 64
HPC = 4                    # heads per core
EC = HPC * DH              # 256 feature cols per core side
NP = 5                     # Chebyshev orders 1..5
NS = S // 128              # 8 s-tiles
NKD = D // 128             # 8 k-tiles over d for QKV
KVW = NP * EC + NP * HPC   # 1300 = 5*256 features + 20 den cols
EPS_DEN = 1e-7
INV_SQRT_D = 1.0 / 8.0     # 1/sqrt(64)
SQ2 = float(np.sqrt(2.0))


def _beta():
    j = np.arange(6, dtype=np.float32)
    alpha = (j + 1.0) ** (-1.5)
    tail = np.flip(np.cumsum(np.flip(alpha)))
    beta = np.concatenate([np.zeros(1, np.float32), tail[1:].astype(np.float32),
                           np.zeros(5, np.float32)])
    return beta / beta.sum()          # (11,); nonzero at 1..5


def _bcast(ap, reps):
    """Broadcast a [..., n] AP to [..., n, reps] via a step-0 inner dim."""
    return bass.AP(tensor=ap.tensor, offset=ap.offset,
                   ap=list(ap.ap) + [[0, reps]])


def _bcast_mid(ap, reps, at):
    """Insert a step-0 dim of size reps at free-dim position `at` (0 = just
    after the partition dim)."""
    new = list(ap.ap)
    new.insert(1 + at, [0, reps])
    return bass.AP(tensor=ap.tensor, offset=ap.offset, ap=new)


def _build():
    nc = bacc.Bacc("TRN2", target_bir_lowering=False, debug=False, num_devices=8)

    XT = nc.dram_tensor("xt", [D, S], BF16, kind="ExternalInput")
    WQKVT = nc.dram_tensor("wqkvt", [D, 3 * EC], BF16, kind="ExternalInput")
    WOUTT = nc.dram_tensor("woutt", [EC, D], BF16, kind="ExternalInput")
    LTB = nc.dram_tensor("ltb", [NP, 128, 128], BF16, kind="ExternalInput")
    LT1 = nc.dram_tensor("lt1", [128, 128], BF16, kind="ExternalInput")
    IDENT = nc.dram_tensor("ident", [128, 128], BF16, kind="ExternalInput")
    BETA5 = nc.dram_tensor("beta5", [128, NP * HPC], BF16, kind="ExternalInput")
    PART = nc.dram_tensor("part", [S, D], BF16, kind="ExternalOutput")

    AX = mybir.AxisListType.X
    OP = mybir.AluOpType
    ACT = mybir.ActivationFunctionType

    with tile.TileContext(nc) as tc:
        with (
            nc.allow_low_precision(reason="bf16 feature pipeline by design"),
            tc.tile_pool(name="persist", bufs=1) as pp,
            tc.tile_pool(name="work", bufs=2) as wp,
            tc.tile_pool(name="site", bufs=3) as sp,
            tc.tile_pool(name="scratch", bufs=1) as xp,
            tc.tile_pool(name="ps_qkv", bufs=1, space="PSUM") as ps_qkv,
            tc.tile_pool(name="ps_kv", bufs=1, space="PSUM") as ps_kv,
            tc.tile_pool(name="ps_t", bufs=1, space="PSUM") as ps_t,
            tc.tile_pool(name="ps_o", bufs=2, space="PSUM") as ps_o,
        ):
            xt = pp.tile([128, NKD, S], BF16)
            wqkvt = pp.tile([128, NKD, 3 * EC], BF16)
            woutt = pp.tile([128, 2, D], BF16)
            ltb = pp.tile([128, NP, 128], BF16)
            lt1 = pp.tile([128, 128], BF16)
            ident = pp.tile([128, 128], BF16)
            beta5 = pp.tile([128, NP, HPC], BF16)

            for k in range(NKD):
                nc.sync.dma_start(out=xt[:, k, :], in_=XT[128 * k:128 * (k + 1), :])
                nc.sync.dma_start(out=wqkvt[:, k, :], in_=WQKVT[128 * k:128 * (k + 1), :])
            for k in range(2):
                nc.sync.dma_start(out=woutt[:, k, :], in_=WOUTT[128 * k:128 * (k + 1), :])
            for p in range(NP):
                nc.sync.dma_start(out=ltb[:, p, :], in_=LTB[p])
            nc.sync.dma_start(out=lt1, in_=LT1.ap())
            nc.sync.dma_start(out=ident, in_=IDENT.ap())
            nc.sync.dma_start(out=beta5.rearrange("a p h -> a (p h)"),
                              in_=BETA5.ap())

            # ---- per-group state (2 groups of 4 s-tiles) ----
            t_g = [None, None]      # [128, 4, NP, 512] T1..T5 for q|k
            tv_g = [None, None]     # [128, 4, NP, 264] Tk*v (+den in 256:260)
            kvt_g = [None, None]    # [128, 4, 1304] cumsum result (bf16)
            vt_g = [None, None]     # [128, 4, 256]
            qs_g = [None, None]     # [128, 4, NP, 8] per-head row sums q|k
            qkv_s = [None] * NS     # per-site PSUM handles
            outh_g = [None, None]

            def qkv_site(i):
                qkv = ps_qkv.tile([128, 768], F32, tag="qkv")
                qkv_s[i] = qkv
                si = slice(128 * i, 128 * (i + 1))
                for k in range(NKD):
                    lhs = xt[:, k, si]
                    nc.tensor.matmul(qkv[:, 0:512], lhs, wqkvt[:, k, 0:512],
                                     start=(k == 0), stop=(k == NKD - 1))
                    nc.tensor.matmul(qkv[:, 512:768], lhs, wqkvt[:, k, 512:768],
                                     start=(k == 0), stop=(k == NKD - 1))

            def evac_qkv_site(i):
                g, j = divmod(i, 4)
                if t_g[g] is None:
                    t_g[g] = wp.tile([128, 4, NP, 512], BF16, tag="t", name="t")
                    vt_g[g] = wp.tile([128, 4, EC + 8], BF16, tag="vt", name="vt")
                qkv = qkv_s[i]
                nc.scalar.copy(out=t_g[g][:, j, 0, :], in_=qkv[:, 0:512])
                nc.scalar.copy(out=vt_g[g][:, j, 0:EC], in_=qkv[:, 512:768])

            def cheb_group(g):
                t = t_g[g]
                vt = vt_g[g]
                tv = wp.tile([128, 4, NP, 264], BF16, tag="tv")
                qs = wp.tile([128, 4, NP, 8], BF16, tag="qs")
                u = xp.tile([128, 4, 512], BF16, tag="u")
                w2 = xp.tile([128, 4, 512], BF16, tag="w2")
                fa = xp.tile([128, 160, 32], BF16, tag="fa")
                fb = xp.tile([128, 160, 16], BF16, tag="fb")
                tv_g[g] = tv
                qs_g[g] = qs
                x1 = t[:, :, 0, :]
                t2, t3, t4, t5 = (t[:, :, p, :] for p in range(1, NP))
                TT = nc.vector.tensor_tensor
                TS = nc.vector.tensor_scalar
                # Chebyshev chain: tensor_scalar hits the 4x DVE mode, STT
                # gets none -> express T3/T5 via TS+TT.
                TT(out=u, in0=x1, in1=x1, op=OP.mult)            # x^2
                TS(out=t2, in0=u, scalar1=2.0, scalar2=-1.0,
                   op0=OP.mult, op1=OP.add)                      # T2
                TS(out=w2, in0=t2, scalar1=2.0, scalar2=-1.0,
                   op0=OP.mult, op1=OP.add)                      # 2T2-1
                TT(out=t3, in0=x1, in1=w2, op=OP.mult)           # T3
                TT(out=u, in0=t2, in1=t2, op=OP.mult)            # T2^2
                TS(out=t4, in0=u, scalar1=2.0, scalar2=-1.0,
                   op0=OP.mult, op1=OP.add)                      # T4
                TS(out=w2, in0=t3, scalar1=2.0, scalar2=0.0,
                   op0=OP.mult, op1=OP.add)                      # 2T3
                TT(out=t5, in0=t2, in1=w2, op=OP.mult)           # 2T2T3
                TT(out=t5, in0=t5, in1=x1, op=OP.subtract)       # T5
                # per-head row sums over d=64 via a TT-add fold tree
                # (plain reduce runs at ~1.1ns/col; folds get the 2x mode)
                tf = t.rearrange("a s p (c e) -> a (s p c) e", e=DH)
                TT(out=fa, in0=tf[:, :, 0:32], in1=tf[:, :, 32:64], op=OP.add)
                TT(out=fb, in0=fa[:, :, 0:16], in1=fa[:, :, 16:32], op=OP.add)
                TT(out=fa[:, :, 0:8], in0=fb[:, :, 0:8], in1=fb[:, :, 8:16],
                   op=OP.add)
                TT(out=fb[:, :, 0:4], in0=fa[:, :, 0:4], in1=fa[:, :, 4:8],
                   op=OP.add)
                TT(out=fa[:, :, 0:2], in0=fb[:, :, 0:2], in1=fb[:, :, 2:4],
                   op=OP.add)
                TT(out=qs.rearrange("a s p c -> a (s p c)"),
                   in0=fa[:, :, 0:1].rearrange("a n e -> a (n e)"),
                   in1=fa[:, :, 1:2].rearrange("a n e -> a (n e)"), op=OP.add)
                # den channels: beta_p * ksum -> tv[..., 256:260]
                TT(out=tv[:, :, :, 256:260], in0=qs[:, :, :, 4:8],
                   in1=_bcast_mid(beta5, 4, 0), op=OP.mult)
                # Tv = Tk * v
                for p in range(NP):
                    TT(out=tv[:, :, p, 0:256], in0=t[:, :, p, 256:512],
                       in1=vt[:, :, 0:EC], op=OP.mult)

            def cumsum_site(i):
                g, j = divmod(i, 4)
                if kvt_g[g] is None:
                    kvt_g[g] = wp.tile([128, 4, 1304], BF16, tag="kvt", name="kvt")
                tv = tv_g[g]
                kvt = kvt_g[g]
                first = (i == 0)
                kv = ps_kv.tile([128, KVW], F32, tag="kv")
                for p in range(NP):
                    nc.tensor.matmul(kv[:, 256 * p:256 * (p + 1)],
                                     ltb[:, p, :], tv[:, j, p, 0:256],
                                     start=True, stop=True)
                nc.tensor.matmul(kv[:, 1280:1300], lt1,
                                 tv[:, j, :, 256:260],
                                 start=True, stop=True)
                # evacuate (+ add running carry, broadcast by Pool from the
                # previous site's partition-0 row)
                if first:
                    nc.vector.tensor_copy(out=kvt[:, j, 0:650], in_=kv[:, 0:650])
                    nc.scalar.copy(out=kvt[:, j, 650:1300], in_=kv[:, 650:1300])
                else:
                    gp, jp = divmod(i - 1, 4)
                    cb = sp.tile([128, 1304], BF16, tag="cb")
                    nc.gpsimd.partition_broadcast(
                        cb[:, 0:1300], kvt_g[gp][0:1, jp, 0:1300], channels=128)
                    nc.vector.tensor_tensor(out=kvt[:, j, 0:1300],
                                            in0=kv[:, 0:1300],
                                            in1=cb[:, 0:1300], op=OP.add)

            def numden_group(g):
                t = t_g[g]
                kvt = kvt_g[g]
                qs = qs_g[g]
                pr = xp.tile([128, 4, NP, EC], BF16, tag="pr")
                s01 = xp.tile([128, 4, EC], BF16, tag="s01")
                s23 = xp.tile([128, 4, EC], BF16, tag="s23")
                numq = xp.tile([128, 4, EC], BF16, tag="numq")
                dpr = xp.tile([128, 4, NP, HPC], F32, tag="dpr")
                den = xp.tile([128, 4, HPC], F32, tag="den")
                rden = xp.tile([128, 4, HPC], F32, tag="rden")
                outh = wp.tile([128, 4, EC], BF16, tag="outh")
                outh_g[g] = outh
                for p in range(NP):
                    nc.vector.tensor_tensor(
                        out=pr[:, :, p, :], in0=t[:, :, p, 0:256],
                        in1=kvt[:, :, 256 * p:256 * (p + 1)], op=OP.mult)
                nc.vector.tensor_add(s01, pr[:, :, 0, :], pr[:, :, 1, :])
                nc.vector.tensor_add(s23, pr[:, :, 2, :], pr[:, :, 3, :])
                nc.vector.tensor_add(s01, s01, pr[:, :, 4, :])
                nc.vector.tensor_tensor(out=numq, in0=s01, in1=s23, op=OP.add)
                # den
                nc.vector.tensor_tensor(
                    out=dpr, in0=qs[:, :, :, 0:4],
                    in1=kvt[:, :, 1280:1300].rearrange("a s (p h) -> a s p h",
                                                       h=HPC),
                    op=OP.mult)
                nc.vector.tensor_reduce(out=den,
                                        in_=dpr.rearrange("a s p h -> a s h p"),
                                        axis=AX, op=OP.add)
                nc.vector.tensor_scalar_add(out=den, in0=den, scalar1=EPS_DEN)
                nc.vector.reciprocal(out=rden, in_=den)
                rdb = xp.tile([128, 4, EC], BF16, tag="rdb")
                nc.scalar.activation(
                    out=rdb.rearrange("a s (h d) -> a s h d", h=HPC),
                    in_=_bcast(rden, DH), func=ACT.Copy)
                nc.vector.tensor_tensor(out=outh, in0=numq, in1=rdb,
                                        op=OP.mult)

            def proj_site(i):
                g, j = divmod(i, 4)
                outh = outh_g[g]
                si = slice(128 * i, 128 * (i + 1))
                tp = ps_t.tile([128, 2, 128], BF16, tag="tp")
                outt = sp.tile([128, 2, 128], BF16, tag="outt")
                for kt in range(2):
                    nc.tensor.transpose(tp[:, kt, :],
                                        outh[:, j, 128 * kt:128 * (kt + 1)],
                                        ident)
                nc.scalar.copy(out=outt, in_=tp)
                outf = sp.tile([128, D], BF16, tag="outf")
                for n in range(2):
                    op_ps = ps_o.tile([128, 512], F32, tag="op")
                    for kt in range(2):
                        nc.tensor.matmul(op_ps, outt[:, kt, :],
                                         woutt[:, kt, 512 * n:512 * (n + 1)],
                                         start=(kt == 0), stop=(kt == 1))
                    if n == 0:
                        nc.vector.tensor_copy(out=outf[:, 0:512], in_=op_ps)
                    else:
                        nc.scalar.copy(out=outf[:, 512:1024], in_=op_ps)
                nc.sync.dma_start(out=PART[si, 0:512], in_=outf[:, 0:512])
                nc.sync.dma_start(out=PART[si, 512:1024], in_=outf[:, 512:1024])

            # ---------------- software-pipelined emission ----------------
            for i in range(4):
                qkv_site(i)
                evac_qkv_site(i)
            cheb_group(0)
            qkv_site(4)
            evac_qkv_site(4)
            cumsum_site(0)
            qkv_site(5)
            evac_qkv_site(5)
            cumsum_site(1)
            qkv_site(6)
            evac_qkv_site(6)
            cumsum_site(2)
            qkv_site(7)
            evac_qkv_site(7)
            cumsum_site(3)
            cheb_group(1)
            numden_group(0)
            cumsum_site(4)
            proj_site(0)
            cumsum_site(5)
            proj_site(1)
            cumsum_site(6)
            proj_site(2)
            cumsum_site(7)
            proj_site(3)
            numden_group(1)
            for i in range(4, 8):
                proj_site(i)

    nc.compile()
    return nc


_NC = None


def _get_nc():
    global _NC
    if _NC is None:
        _NC = _build()
    return _NC


def _stage_inputs(x, W_in, W_out):
    import ml_dtypes
    bf = ml_dtypes.bfloat16
    beta = _beta()
    tri = np.tril(np.ones((128, 128), np.float32))
    ltb = np.stack([beta[p] * tri for p in range(1, 6)]).astype(bf)
    lt1 = tri.astype(bf)
    ident = np.eye(128, dtype=bf)
    beta5 = np.tile(np.repeat(beta[1:6], HPC)[None, :], (128, 1)).astype(bf)
    in_maps = []
    for c in range(8):
        b, hb = divmod(c, 4)
        rs = slice(256 * hb, 256 * (hb + 1))
        wq = W_in[0 * D + 256 * hb:0 * D + 256 * (hb + 1)] * INV_SQRT_D
        wk = W_in[1 * D + 256 * hb:1 * D + 256 * (hb + 1)] * INV_SQRT_D
        wv = W_in[2 * D + 256 * hb:2 * D + 256 * (hb + 1)]
        wqkvt = np.ascontiguousarray(
            np.concatenate([wq, wk, wv], axis=0).T).astype(bf)
        xrev = x[b].T.reshape(D, NS, 128)[:, :, ::-1].reshape(D, S)
        in_maps.append({
            "xt": np.ascontiguousarray(xrev).astype(bf),
            "wqkvt": wqkvt,
            "woutt": np.ascontiguousarray(W_out[:, rs].T).astype(bf),
            "ltb": ltb,
            "lt1": lt1,
            "ident": ident,
            "beta5": beta5,
        })
    return in_maps


def _gather(results):
    out = np.zeros((B, S, D), dtype=np.float32)
    for c in range(8):
        part = results[c]["part"].astype(np.float32)
        part = part.reshape(NS, 128, D)[:, ::-1, :].reshape(S, D)
        out[c // 4] += part
    return out


def kernel(x, W_in, W_out):
    from concourse.bass_utils import run_bass_kernel_spmd

    x = np.asarray(x, dtype=np.float32)
    W_in = np.asarray(W_in, dtype=np.float32)
    W_out = np.asarray(W_out, dtype=np.float32)
    nc = _get_nc()
    in_maps = _stage_inputs(x, W_in, W_out)
    res = run_bass_kernel_spmd(nc, in_maps, core_ids=list(range(8)))
    return _gather(res.results)


# revision 24
# speedup vs baseline: 1.1791x; 1.1791x over previous
"""Trainium2 Bass kernel for CollapsedPBFA (collapsed Chebyshev linear attention).

Full-input contract: kernel(x, W_in, W_out) -> (B, S, D) float32.

Sharding: 8 cores = (batch b in {0,1}) x (head-block hb in {0..3} of 4 heads).
QKV is column-parallel per head block; the output projection is row-parallel
(each core computes a partial (S, D) product over its 256 hidden columns) and
the host sums per-core partials per batch element.

v2 layout/engine plan (all bf16 operands, fp32 PSUM):
  - beta is nonzero only for Chebyshev orders 1..5; beta_p is folded into the
    per-p lower-triangular cumsum constants (features) and applied to the
    den channels via a tiny broadcast multiply (den goes through an unscaled
    triangle).
  - kv PSUM layout per s-tile: [p0|p1|p2|p3|p4|den20] at 256-col offsets
    (1300 f32 = 3 banks); carry is 3 banked rank-1 matmuls (512/512/276).
  - Chebyshev chain splits work: Act takes single-input affine ops
    (copies, u-1), DVE takes the tensor-tensor chain at 4-s-tile granularity,
    Pool takes medium copies.  kvt evacuation is split DVE/Act/Pool per site
    to shorten the serial tri->evac->tri carry chain.
  - PE emission is software-pipelined (QKV / cumsum / transpose+proj
    interleaved) to avoid gaps (PE p-state ramp: gaps halve the clock).
  - Output PART is bf16 (host upcasts and sums), halving output DMA.
"""

import sys

for _p in ("/opt/trn_rl_repo", "/root/.axon_site/_ro/trn_rl_repo"):
    if _p not in sys.path:
        sys.path.append(_p)

import os

import numpy as np

import concourse.bacc as bacc
import concourse.bass as bass
import concourse.tile as tile
from concourse import mybir

if os.environ.get("LDWOPT", "0") == "1":
    import concourse.bass_utils as _bu

    if not getattr(_bu, "_ldwopt_patched", False):
        _orig_run_command = _bu.run_command

        def _run_command_ldwopt(cmd, *a, **kw):
            cmd = ["--enable-ldw-opt=true" if c == "--enable-ldw-opt=false" else c
                   for c in cmd]
            return _orig_run_command(cmd, *a, **kw)

        _bu.run_command = _run_command_ldwopt
        _bu._ldwopt_patched = True

F32 = mybir.dt.float32
BF16 = mybir.dt.bfloat16

B, S, D = 2, 1024, 1024
H, DH = 16, 64
HPC = 4                    # heads per core
EC = HPC * DH              # 256 feature cols per core side
NP = 5                     # Chebyshev orders 1..5
NS = S // 128              # 8 s-tiles
NKD = D // 128             # 8 k-tiles over d for QKV
KVW = NP * EC + NP * HPC   # 1300 = 5*256 features + 20 den cols
EPS_DEN = 1e-7
INV_SQRT_D = 1.0 / 8.0     # 1/sqrt(64)
SQ2 = float(np.sqrt(2.0))


def _beta():
    j = np.arange(6, dtype=np.float32)
    alpha = (j + 1.0) ** (-1.5)
    tail = np.flip(np.cumsum(np.flip(alpha)))
    beta = np.concatenate([np.zeros(1, np.float32), tail[1:].astype(np.float32),
                           np.zeros(5, np.float32)])
    return beta / beta.sum()          # (11,); nonzero at 1..5


def _bcast(ap, reps):
    """Broadcast a [..., n] AP to [..., n, reps] via a step-0 inner dim."""
    return bass.AP(tensor=ap.tensor, offset=ap.offset,
                   ap=list(ap.ap) + [[0, reps]])


def _bcast_mid(ap, reps, at):
    """Insert a step-0 dim of size reps at free-dim position `at` (0 = just
    after the partition dim)."""
    new = list(ap.ap)
    new.insert(1 + at, [0, reps])
    return bass.AP(tensor=ap.tensor, offset=ap.offset, ap=new)


def _build():
    nc = bacc.Bacc("TRN2", target_bir_lowering=False, debug=False, num_devices=8)

    XT = nc.dram_tensor("xt", [D, S], BF16, kind="ExternalInput")
    WQKVT = nc.dram_tensor("wqkvt", [D, 3 * EC], BF16, kind="ExternalInput")
    WOUTT = nc.dram_tensor("woutt", [EC, D], BF16, kind="ExternalInput")
    LTB = nc.dram_tensor("ltb", [NP, 128, 128], BF16, kind="ExternalInput")
    LT1 = nc.dram_tensor("lt1", [128, 128], BF16, kind="ExternalInput")
    IDENT = nc.dram_tensor("ident", [128, 128], BF16, kind="ExternalInput")
    BETA5 = nc.dram_tensor("beta5", [128, NP * HPC], BF16, kind="ExternalInput")
    PART = nc.dram_tensor("part", [S, D], BF16, kind="ExternalOutput")

    AX = mybir.AxisListType.X
    OP = mybir.AluOpType
    ACT = mybir.ActivationFunctionType

    with tile.TileContext(nc) as tc:
        with (
            nc.allow_low_precision(reason="bf16 feature pipeline by design"),
            tc.tile_pool(name="persist", bufs=1) as pp,
            tc.tile_pool(name="work", bufs=2) as wp,
            tc.tile_pool(name="site", bufs=3) as sp,
            tc.tile_pool(name="scratch", bufs=1) as xp,
            tc.tile_pool(name="ps_qkv", bufs=1, space="PSUM") as ps_qkv,
            tc.tile_pool(name="ps_kv", bufs=1, space="PSUM") as ps_kv,
            tc.tile_pool(name="ps_t", bufs=1, space="PSUM") as ps_t,
            tc.tile_pool(name="ps_o", bufs=1, space="PSUM") as ps_o,
        ):
            xt = pp.tile([128, NKD, S], BF16)
            wqkvt = pp.tile([128, NKD, 3 * EC], BF16)
            woutt = pp.tile([128, 2, D], BF16)
            ltb = pp.tile([128, NP, 128], BF16)
            lt1 = pp.tile([128, 128], BF16)
            ident = pp.tile([128, 128], BF16)
            beta5 = pp.tile([128, NP, HPC], BF16)

            for k in range(NKD):
                nc.sync.dma_start(out=xt[:, k, :], in_=XT[128 * k:128 * (k + 1), :])
                nc.sync.dma_start(out=wqkvt[:, k, :], in_=WQKVT[128 * k:128 * (k + 1), :])
            for k in range(2):
                nc.sync.dma_start(out=woutt[:, k, :], in_=WOUTT[128 * k:128 * (k + 1), :])
            for p in range(NP):
                nc.sync.dma_start(out=ltb[:, p, :], in_=LTB[p])
            nc.sync.dma_start(out=lt1, in_=LT1.ap())
            nc.sync.dma_start(out=ident, in_=IDENT.ap())
            nc.sync.dma_start(out=beta5.rearrange("a p h -> a (p h)"),
                              in_=BETA5.ap())

            # ---- per-group state (2 groups of 4 s-tiles) ----
            t_g = [None, None]      # [128, 4, NP, 512] T1..T5 for q|k
            tv_g = [None, None]     # [128, 4, NP, 264] Tk*v (+den in 256:260)
            kvt_g = [None, None]    # [128, 4, 1304] cumsum result (bf16)
            vt_g = [None, None]     # [128, 4, 256]
            qs_g = [None, None]     # [128, 4, NP, 8] per-head row sums q|k
            qkv_s = [None] * NS     # per-site PSUM handles
            outh_g = [None, None]

            def qkv_site(i):
                qkv = ps_qkv.tile([128, 768], F32, tag="qkv")
                qkv_s[i] = qkv
                si = slice(128 * i, 128 * (i + 1))
                for k in range(NKD):
                    lhs = xt[:, k, si]
                    nc.tensor.matmul(qkv[:, 0:512], lhs, wqkvt[:, k, 0:512],
                                     start=(k == 0), stop=(k == NKD - 1))
                    mm = nc.tensor.matmul(qkv[:, 512:768], lhs,
                                          wqkvt[:, k, 512:768],
                                          start=(k == 0), stop=(k == NKD - 1))
                    mm.ldweights = False  # same stationary tile as previous mm

            def evac_qkv_site(i):
                g, j = divmod(i, 4)
                if t_g[g] is None:
                    t_g[g] = wp.tile([128, 4, NP, 512], BF16, tag="t", name="t")
                    vt_g[g] = wp.tile([128, 4, EC + 8], BF16, tag="vt", name="vt")
                qkv = qkv_s[i]
                nc.scalar.copy(out=t_g[g][:, j, 0, :], in_=qkv[:, 0:512])
                nc.scalar.copy(out=vt_g[g][:, j, 0:EC], in_=qkv[:, 512:768])

            def cheb_group(g):
                t = t_g[g]
                vt = vt_g[g]
                tv = wp.tile([128, 4, NP, 264], BF16, tag="tv")
                qs = wp.tile([128, 4, NP, 8], BF16, tag="qs")
                u = xp.tile([128, 4, 512], BF16, tag="u")
                w2 = xp.tile([128, 4, 512], BF16, tag="w2")
                fa = xp.tile([128, 160, 32], BF16, tag="fa")
                fb = xp.tile([128, 160, 16], BF16, tag="fb")
                tv_g[g] = tv
                qs_g[g] = qs
                x1 = t[:, :, 0, :]
                t2, t3, t4, t5 = (t[:, :, p, :] for p in range(1, NP))
                TT = nc.vector.tensor_tensor
                TS = nc.vector.tensor_scalar
                # Chebyshev chain: tensor_scalar hits the 4x DVE mode, STT
                # gets none -> express T3/T5 via TS+TT.
                TT(out=u, in0=x1, in1=x1, op=OP.mult)            # x^2
                TS(out=t2, in0=u, scalar1=2.0, scalar2=-1.0,
                   op0=OP.mult, op1=OP.add)                      # T2
                TS(out=w2, in0=t2, scalar1=2.0, scalar2=-1.0,
                   op0=OP.mult, op1=OP.add)                      # 2T2-1
                TT(out=t3, in0=x1, in1=w2, op=OP.mult)           # T3
                TT(out=u, in0=t2, in1=t2, op=OP.mult)            # T2^2
                TS(out=t4, in0=u, scalar1=2.0, scalar2=-1.0,
                   op0=OP.mult, op1=OP.add)                      # T4
                TS(out=w2, in0=t3, scalar1=2.0, scalar2=0.0,
                   op0=OP.mult, op1=OP.add)                      # 2T3
                TT(out=t5, in0=t2, in1=w2, op=OP.mult)           # 2T2T3
                TT(out=t5, in0=t5, in1=x1, op=OP.subtract)       # T5
                # per-head row sums over d=64 via a TT-add fold tree
                # (plain reduce runs at ~1.1ns/col; folds get the 2x mode)
                tf = t.rearrange("a s p (c e) -> a (s p c) e", e=DH)
                TT(out=fa, in0=tf[:, :, 0:32], in1=tf[:, :, 32:64], op=OP.add)
                TT(out=fb, in0=fa[:, :, 0:16], in1=fa[:, :, 16:32], op=OP.add)
                TT(out=fa[:, :, 0:8], in0=fb[:, :, 0:8], in1=fb[:, :, 8:16],
                   op=OP.add)
                TT(out=fb[:, :, 0:4], in0=fa[:, :, 0:4], in1=fa[:, :, 4:8],
                   op=OP.add)
                TT(out=fa[:, :, 0:2], in0=fb[:, :, 0:2], in1=fb[:, :, 2:4],
                   op=OP.add)
                TT(out=qs.rearrange("a s p c -> a (s p c)"),
                   in0=fa[:, :, 0:1].rearrange("a n e -> a (n e)"),
                   in1=fa[:, :, 1:2].rearrange("a n e -> a (n e)"), op=OP.add)
                # den channels: beta_p * ksum -> tv[..., 256:260]
                TT(out=tv[:, :, :, 256:260], in0=qs[:, :, :, 4:8],
                   in1=_bcast_mid(beta5, 4, 0), op=OP.mult)
                # Tv = Tk * v
                for p in range(NP):
                    TT(out=tv[:, :, p, 0:256], in0=t[:, :, p, 256:512],
                       in1=vt[:, :, 0:EC], op=OP.mult)

            def cumsum_site(i):
                g, j = divmod(i, 4)
                if kvt_g[g] is None:
                    kvt_g[g] = wp.tile([128, 4, 1304], BF16, tag="kvt", name="kvt")
                tv = tv_g[g]
                kvt = kvt_g[g]
                first = (i == 0)
                kv = ps_kv.tile([128, KVW], F32, tag="kv")
                for p in range(NP):
                    nc.tensor.matmul(kv[:, 256 * p:256 * (p + 1)],
                                     ltb[:, p, :], tv[:, j, p, 0:256],
                                     start=True, stop=True)
                nc.tensor.matmul(kv[:, 1280:1300], lt1,
                                 tv[:, j, :, 256:260],
                                 start=True, stop=True)
                # evacuate (+ add running carry, broadcast by Pool from the
                # previous site's partition-0 row)
                if first:
                    nc.vector.tensor_copy(out=kvt[:, j, 0:650], in_=kv[:, 0:650])
                    nc.scalar.copy(out=kvt[:, j, 650:1300], in_=kv[:, 650:1300])
                else:
                    gp, jp = divmod(i - 1, 4)
                    cb = sp.tile([128, 1304], BF16, tag="cb")
                    nc.gpsimd.partition_broadcast(
                        cb[:, 0:1300], kvt_g[gp][0:1, jp, 0:1300], channels=128)
                    nc.vector.tensor_tensor(out=kvt[:, j, 0:1300],
                                            in0=kv[:, 0:1300],
                                            in1=cb[:, 0:1300], op=OP.add)

            def numden_group(g):
                t = t_g[g]
                kvt = kvt_g[g]
                qs = qs_g[g]
                pr = xp.tile([128, 4, NP, EC], BF16, tag="pr")
                s01 = xp.tile([128, 4, EC], BF16, tag="s01")
                s23 = xp.tile([128, 4, EC], BF16, tag="s23")
                numq = xp.tile([128, 4, EC], BF16, tag="numq")
                dpr = xp.tile([128, 4, NP, HPC], F32, tag="dpr")
                den = xp.tile([128, 4, HPC], F32, tag="den")
                rden = xp.tile([128, 4, HPC], F32, tag="rden")
                outh = wp.tile([128, 4, EC], BF16, tag="outh")
                outh_g[g] = outh
                for p in range(NP):
                    nc.vector.tensor_tensor(
                        out=pr[:, :, p, :], in0=t[:, :, p, 0:256],
                        in1=kvt[:, :, 256 * p:256 * (p + 1)], op=OP.mult)
                nc.vector.tensor_add(s01, pr[:, :, 0, :], pr[:, :, 1, :])
                nc.vector.tensor_add(s23, pr[:, :, 2, :], pr[:, :, 3, :])
                nc.vector.tensor_add(s01, s01, pr[:, :, 4, :])
                nc.vector.tensor_tensor(out=numq, in0=s01, in1=s23, op=OP.add)
                # den
                nc.vector.tensor_tensor(
                    out=dpr, in0=qs[:, :, :, 0:4],
                    in1=kvt[:, :, 1280:1300].rearrange("a s (p h) -> a s p h",
                                                       h=HPC),
                    op=OP.mult)
                nc.vector.tensor_reduce(out=den,
                                        in_=dpr.rearrange("a s p h -> a s h p"),
                                        axis=AX, op=OP.add)
                nc.vector.tensor_scalar_add(out=den, in0=den, scalar1=EPS_DEN)
                nc.vector.reciprocal(out=rden, in_=den)
                rdb = xp.tile([128, 4, EC], BF16, tag="rdb")
                nc.scalar.activation(
                    out=rdb.rearrange("a s (h d) -> a s h d", h=HPC),
                    in_=_bcast(rden, DH), func=ACT.Copy)
                nc.vector.tensor_tensor(out=outh, in0=numq, in1=rdb,
                                        op=OP.mult)

            def proj_site(i):
                g, j = divmod(i, 4)
                outh = outh_g[g]
                si = slice(128 * i, 128 * (i + 1))
                tp = ps_t.tile([128, 2, 128], BF16, tag="tp")
                outt = sp.tile([128, 2, 128], BF16, tag="outt")
                for kt in range(2):
                    nc.tensor.transpose(tp[:, kt, :],
                                        outh[:, j, 128 * kt:128 * (kt + 1)],
                                        ident)
                nc.scalar.copy(out=outt, in_=tp)
                outf = sp.tile([128, D], BF16, tag="outf")
                op0 = ps_o.tile([128, 512], F32, tag="op0")
                op1 = ps_o.tile([128, 512], F32, tag="op1")
                for kt in range(2):
                    nc.tensor.matmul(op0, outt[:, kt, :],
                                     woutt[:, kt, 0:512],
                                     start=(kt == 0), stop=(kt == 1))
                    mm = nc.tensor.matmul(op1, outt[:, kt, :],
                                          woutt[:, kt, 512:1024],
                                          start=(kt == 0), stop=(kt == 1))
                    mm.ldweights = False  # shares stationary outt[:, kt]
                nc.vector.tensor_copy(out=outf[:, 0:512], in_=op0)
                nc.scalar.copy(out=outf[:, 512:1024], in_=op1)
                nc.sync.dma_start(out=PART[si, 0:512], in_=outf[:, 0:512])
                nc.sync.dma_start(out=PART[si, 512:1024], in_=outf[:, 512:1024])

            # ---------------- software-pipelined emission ----------------
            for i in range(4):
                qkv_site(i)
                evac_qkv_site(i)
            cheb_group(0)
            qkv_site(4)
            evac_qkv_site(4)
            cumsum_site(0)
            qkv_site(5)
            evac_qkv_site(5)
            cumsum_site(1)
            qkv_site(6)
            evac_qkv_site(6)
            cumsum_site(2)
            qkv_site(7)
            evac_qkv_site(7)
            cumsum_site(3)
            cheb_group(1)
            numden_group(0)
            cumsum_site(4)
            proj_site(0)
            cumsum_site(5)
            proj_site(1)
            cumsum_site(6)
            proj_site(2)
            cumsum_site(7)
            proj_site(3)
            numden_group(1)
            for i in range(4, 8):
                proj_site(i)

    nc.compile()
    return nc


_NC = None


def _get_nc():
    global _NC
    if _NC is None:
        _NC = _build()
    return _NC


def _stage_inputs(x, W_in, W_out):
    import ml_dtypes
    bf = ml_dtypes.bfloat16
    beta = _beta()
    tri = np.tril(np.ones((128, 128), np.float32))
    ltb = np.stack([beta[p] * tri for p in range(1, 6)]).astype(bf)
    lt1 = tri.astype(bf)
    ident = np.eye(128, dtype=bf)
    beta5 = np.tile(np.repeat(beta[1:6], HPC)[None, :], (128, 1)).astype(bf)
    in_maps = []
    for c in range(8):
        b, hb = divmod(c, 4)
        rs = slice(256 * hb, 256 * (hb + 1))
        wq = W_in[0 * D + 256 * hb:0 * D + 256 * (hb + 1)] * INV_SQRT_D
        wk = W_in[1 * D + 256 * hb:1 * D + 256 * (hb + 1)] * INV_SQRT_D
        wv = W_in[2 * D + 256 * hb:2 * D + 256 * (hb + 1)]
        wqkvt = np.ascontiguousarray(
            np.concatenate([wq, wk, wv], axis=0).T).astype(bf)
        xrev = x[b].T.reshape(D, NS, 128)[:, :, ::-1].reshape(D, S)
        in_maps.append({
            "xt": np.ascontiguousarray(xrev).astype(bf),
            "wqkvt": wqkvt,
            "woutt": np.ascontiguousarray(W_out[:, rs].T).astype(bf),
            "ltb": ltb,
            "lt1": lt1,
            "ident": ident,
            "beta5": beta5,
        })
    return in_maps


def _gather(results):
    out = np.zeros((B, S, D), dtype=np.float32)
    for c in range(8):
        part = results[c]["part"].astype(np.float32)
        part = part.reshape(NS, 128, D)[:, ::-1, :].reshape(S, D)
        out[c // 4] += part
    return out


def kernel(x, W_in, W_out):
    from concourse.bass_utils import run_bass_kernel_spmd

    x = np.asarray(x, dtype=np.float32)
    W_in = np.asarray(W_in, dtype=np.float32)
    W_out = np.asarray(W_out, dtype=np.float32)
    nc = _get_nc()
    in_maps = _stage_inputs(x, W_in, W_out)
    res = run_bass_kernel_spmd(nc, in_maps, core_ids=list(range(8)))
    return _gather(res.results)


# revision 27
# speedup vs baseline: 1.2020x; 1.0195x over previous
"""Trainium2 Bass kernel for CollapsedPBFA (collapsed Chebyshev linear attention).

Full-input contract: kernel(x, W_in, W_out) -> (B, S, D) float32.

Sharding: 8 cores = (batch b in {0,1}) x (head-block hb in {0..3} of 4 heads).
QKV is column-parallel per head block; the output projection is row-parallel
(each core computes a partial (S, D) product over its 256 hidden columns) and
the host sums per-core partials per batch element.

v2 layout/engine plan (all bf16 operands, fp32 PSUM):
  - beta is nonzero only for Chebyshev orders 1..5; beta_p is folded into the
    per-p lower-triangular cumsum constants (features) and applied to the
    den channels via a tiny broadcast multiply (den goes through an unscaled
    triangle).
  - kv PSUM layout per s-tile: [p0|p1|p2|p3|p4|den20] at 256-col offsets
    (1300 f32 = 3 banks); carry is 3 banked rank-1 matmuls (512/512/276).
  - Chebyshev chain splits work: Act takes single-input affine ops
    (copies, u-1), DVE takes the tensor-tensor chain at 4-s-tile granularity,
    Pool takes medium copies.  kvt evacuation is split DVE/Act/Pool per site
    to shorten the serial tri->evac->tri carry chain.
  - PE emission is software-pipelined (QKV / cumsum / transpose+proj
    interleaved) to avoid gaps (PE p-state ramp: gaps halve the clock).
  - Output PART is bf16 (host upcasts and sums), halving output DMA.
"""

import sys

for _p in ("/opt/trn_rl_repo", "/root/.axon_site/_ro/trn_rl_repo"):
    if _p not in sys.path:
        sys.path.append(_p)

import os

import numpy as np

import concourse.bacc as bacc
import concourse.bass as bass
import concourse.tile as tile
from concourse import mybir

if os.environ.get("LDWOPT", "0") == "1":
    import concourse.bass_utils as _bu

    if not getattr(_bu, "_ldwopt_patched", False):
        _orig_run_command = _bu.run_command

        def _run_command_ldwopt(cmd, *a, **kw):
            cmd = ["--enable-ldw-opt=true" if c == "--enable-ldw-opt=false" else c
                   for c in cmd]
            return _orig_run_command(cmd, *a, **kw)

        _bu.run_command = _run_command_ldwopt
        _bu._ldwopt_patched = True

F32 = mybir.dt.float32
BF16 = mybir.dt.bfloat16

B, S, D = 2, 1024, 1024
H, DH = 16, 64
HPC = 4                    # heads per core
EC = HPC * DH              # 256 feature cols per core side
NP = 5                     # Chebyshev orders 1..5
NS = S // 128              # 8 s-tiles
NKD = D // 128             # 8 k-tiles over d for QKV
KVW = NP * EC + NP * HPC   # 1300 = 5*256 features + 20 den cols
EPS_DEN = 1e-7
INV_SQRT_D = 1.0 / 8.0     # 1/sqrt(64)
SQ2 = float(np.sqrt(2.0))


def _beta():
    j = np.arange(6, dtype=np.float32)
    alpha = (j + 1.0) ** (-1.5)
    tail = np.flip(np.cumsum(np.flip(alpha)))
    beta = np.concatenate([np.zeros(1, np.float32), tail[1:].astype(np.float32),
                           np.zeros(5, np.float32)])
    return beta / beta.sum()          # (11,); nonzero at 1..5


def _bcast(ap, reps):
    """Broadcast a [..., n] AP to [..., n, reps] via a step-0 inner dim."""
    return bass.AP(tensor=ap.tensor, offset=ap.offset,
                   ap=list(ap.ap) + [[0, reps]])


def _bcast_mid(ap, reps, at):
    """Insert a step-0 dim of size reps at free-dim position `at` (0 = just
    after the partition dim)."""
    new = list(ap.ap)
    new.insert(1 + at, [0, reps])
    return bass.AP(tensor=ap.tensor, offset=ap.offset, ap=new)


def _build():
    nc = bacc.Bacc("TRN2", target_bir_lowering=False, debug=False, num_devices=8)

    XT = nc.dram_tensor("xt", [D, S], BF16, kind="ExternalInput")
    WQKVT = nc.dram_tensor("wqkvt", [D, 3 * EC], BF16, kind="ExternalInput")
    WOUTT = nc.dram_tensor("woutt", [EC, D], BF16, kind="ExternalInput")
    LTB = nc.dram_tensor("ltb", [NP, 128, 128], BF16, kind="ExternalInput")
    LT1 = nc.dram_tensor("lt1", [128, 128], BF16, kind="ExternalInput")
    IDENT = nc.dram_tensor("ident", [128, 128], BF16, kind="ExternalInput")
    BETA5 = nc.dram_tensor("beta5", [128, NP * HPC], BF16, kind="ExternalInput")
    PART = nc.dram_tensor("part", [S, D], BF16, kind="ExternalOutput")

    AX = mybir.AxisListType.X
    OP = mybir.AluOpType
    ACT = mybir.ActivationFunctionType

    with tile.TileContext(nc) as tc:
        with (
            nc.allow_low_precision(reason="bf16 feature pipeline by design"),
            tc.tile_pool(name="persist", bufs=1) as pp,
            tc.tile_pool(name="work", bufs=2) as wp,
            tc.tile_pool(name="site", bufs=3) as sp,
            tc.tile_pool(name="scratch", bufs=1) as xp,
            tc.tile_pool(name="ps_qkv", bufs=1, space="PSUM") as ps_qkv,
            tc.tile_pool(name="ps_kv", bufs=1, space="PSUM") as ps_kv,
            tc.tile_pool(name="ps_t", bufs=1, space="PSUM") as ps_t,
            tc.tile_pool(name="ps_o", bufs=1, space="PSUM") as ps_o,
        ):
            xt = pp.tile([128, NKD, S], BF16)
            wqkvt = pp.tile([128, NKD, 3 * EC], BF16)
            woutt = pp.tile([128, 2, D], BF16)
            ltb = pp.tile([128, NP, 128], BF16)
            lt1 = pp.tile([128, 128], BF16)
            ident = pp.tile([128, 128], BF16)
            beta5 = pp.tile([128, NP, HPC], BF16)

            for k in range(NKD):
                nc.sync.dma_start(out=xt[:, k, :], in_=XT[128 * k:128 * (k + 1), :])
                nc.sync.dma_start(out=wqkvt[:, k, :], in_=WQKVT[128 * k:128 * (k + 1), :])
            for k in range(2):
                nc.sync.dma_start(out=woutt[:, k, :], in_=WOUTT[128 * k:128 * (k + 1), :])
            for p in range(NP):
                nc.sync.dma_start(out=ltb[:, p, :], in_=LTB[p])
            nc.sync.dma_start(out=lt1, in_=LT1.ap())
            nc.sync.dma_start(out=ident, in_=IDENT.ap())
            nc.sync.dma_start(out=beta5.rearrange("a p h -> a (p h)"),
                              in_=BETA5.ap())

            # ---- per-group state (4 groups of 2 s-tiles) ----
            NG, GW = 4, 2
            t_g = [None] * NG       # [128, GW, NP, 512] T1..T5 for q|k
            tv_g = [None] * NG      # [128, GW, NP, 264] Tk*v (+den in 256:260)
            kvt_g = [None] * NG     # [128, GW, 1304] cumsum result (bf16)
            vt_g = [None] * NG      # [128, GW, 264]
            qs_g = [None] * NG      # [128, GW, NP, 8] per-head row sums q|k
            qkv_s = [None] * NS     # per-site PSUM handles
            outh_g = [None] * NG

            def qkv_site(i):
                qkv = ps_qkv.tile([128, 768], F32, tag="qkv")
                qkv_s[i] = qkv
                si = slice(128 * i, 128 * (i + 1))
                for k in range(NKD):
                    lhs = xt[:, k, si]
                    nc.tensor.matmul(qkv[:, 0:512], lhs, wqkvt[:, k, 0:512],
                                     start=(k == 0), stop=(k == NKD - 1))
                    nc.tensor.matmul(qkv[:, 512:768], lhs, wqkvt[:, k, 512:768],
                                     start=(k == 0), stop=(k == NKD - 1))
                g, j = divmod(i, GW)
                if t_g[g] is None:
                    t_g[g] = wp.tile([128, GW, NP, 512], BF16, tag="t", name="t")
                    vt_g[g] = wp.tile([128, GW, EC + 8], BF16, tag="vt",
                                      name="vt")
                nc.scalar.copy(out=t_g[g][:, j, 0, :], in_=qkv[:, 0:512])
                nc.scalar.copy(out=vt_g[g][:, j, 0:EC], in_=qkv[:, 512:768])

            def cheb_group(g):
                t = t_g[g]
                vt = vt_g[g]
                tv = wp.tile([128, GW, NP, 264], BF16, tag="tv")
                qs = wp.tile([128, GW, NP, 8], BF16, tag="qs")
                u = xp.tile([128, GW, 512], BF16, tag="u")
                w2 = xp.tile([128, GW, 512], BF16, tag="w2")
                fa = xp.tile([128, GW * 40, 32], BF16, tag="fa")
                fb = xp.tile([128, GW * 40, 16], BF16, tag="fb")
                tv_g[g] = tv
                qs_g[g] = qs
                x1 = t[:, :, 0, :]
                t2, t3, t4, t5 = (t[:, :, p, :] for p in range(1, NP))
                TT = nc.vector.tensor_tensor
                TS = nc.vector.tensor_scalar
                # Chebyshev chain: tensor_scalar hits the 4x DVE mode, STT
                # gets none -> express T3/T5 via TS+TT.
                TT(out=u, in0=x1, in1=x1, op=OP.mult)            # x^2
                TS(out=t2, in0=u, scalar1=2.0, scalar2=-1.0,
                   op0=OP.mult, op1=OP.add)                      # T2
                TS(out=w2, in0=t2, scalar1=2.0, scalar2=-1.0,
                   op0=OP.mult, op1=OP.add)                      # 2T2-1
                TT(out=t3, in0=x1, in1=w2, op=OP.mult)           # T3
                TT(out=u, in0=t2, in1=t2, op=OP.mult)            # T2^2
                TS(out=t4, in0=u, scalar1=2.0, scalar2=-1.0,
                   op0=OP.mult, op1=OP.add)                      # T4
                TS(out=w2, in0=t3, scalar1=2.0, scalar2=0.0,
                   op0=OP.mult, op1=OP.add)                      # 2T3
                TT(out=t5, in0=t2, in1=w2, op=OP.mult)           # 2T2T3
                TT(out=t5, in0=t5, in1=x1, op=OP.subtract)       # T5
                # Tv = Tk * v first: the five feature cumsum matmuls only
                # need these, so the PE can start before the fold tree runs
                for p in range(NP):
                    TT(out=tv[:, :, p, 0:256], in0=t[:, :, p, 256:512],
                       in1=vt[:, :, 0:EC], op=OP.mult)
                # per-head row sums over d=64 via a TT-add fold tree
                # (plain reduce runs at ~1.1ns/col; folds get the 2x mode)
                tf = t.rearrange("a s p (c e) -> a (s p c) e", e=DH)
                TT(out=fa, in0=tf[:, :, 0:32], in1=tf[:, :, 32:64], op=OP.add)
                TT(out=fb, in0=fa[:, :, 0:16], in1=fa[:, :, 16:32], op=OP.add)
                TT(out=fa[:, :, 0:8], in0=fb[:, :, 0:8], in1=fb[:, :, 8:16],
                   op=OP.add)
                TT(out=fb[:, :, 0:4], in0=fa[:, :, 0:4], in1=fa[:, :, 4:8],
                   op=OP.add)
                TT(out=fa[:, :, 0:2], in0=fb[:, :, 0:2], in1=fb[:, :, 2:4],
                   op=OP.add)
                TT(out=qs.rearrange("a s p c -> a (s p c)"),
                   in0=fa[:, :, 0:1].rearrange("a n e -> a (n e)"),
                   in1=fa[:, :, 1:2].rearrange("a n e -> a (n e)"), op=OP.add)
                # den channels: beta_p * ksum -> tv[..., 256:260]
                TT(out=tv[:, :, :, 256:260], in0=qs[:, :, :, 4:8],
                   in1=_bcast_mid(beta5, GW, 0), op=OP.mult)

            def cumsum_site(i):
                g, j = divmod(i, GW)
                if kvt_g[g] is None:
                    kvt_g[g] = wp.tile([128, GW, 1304], BF16, tag="kvt",
                                       name="kvt")
                tv = tv_g[g]
                kvt = kvt_g[g]
                first = (i == 0)
                kv = ps_kv.tile([128, KVW], F32, tag="kv")
                for p in range(NP):
                    nc.tensor.matmul(kv[:, 256 * p:256 * (p + 1)],
                                     ltb[:, p, :], tv[:, j, p, 0:256],
                                     start=True, stop=True)
                nc.tensor.matmul(kv[:, 1280:1300], lt1,
                                 tv[:, j, :, 256:260],
                                 start=True, stop=True)
                # evacuate (+ add running carry, broadcast by Pool from the
                # previous site's partition-0 row)
                if first:
                    nc.vector.tensor_copy(out=kvt[:, j, 0:650], in_=kv[:, 0:650])
                    nc.scalar.copy(out=kvt[:, j, 650:1300], in_=kv[:, 650:1300])
                else:
                    gp, jp = divmod(i - 1, GW)
                    cb = sp.tile([128, 1304], BF16, tag="cb")
                    nc.gpsimd.partition_broadcast(
                        cb[:, 0:1300], kvt_g[gp][0:1, jp, 0:1300], channels=128)
                    nc.vector.tensor_tensor(out=kvt[:, j, 0:1300],
                                            in0=kv[:, 0:1300],
                                            in1=cb[:, 0:1300], op=OP.add)

            def numden_group(g):
                t = t_g[g]
                kvt = kvt_g[g]
                qs = qs_g[g]
                pr = xp.tile([128, GW, NP, EC], BF16, tag="pr")
                s01 = xp.tile([128, GW, EC], BF16, tag="s01")
                s23 = xp.tile([128, GW, EC], BF16, tag="s23")
                numq = xp.tile([128, GW, EC], BF16, tag="numq")
                dpr = xp.tile([128, GW, NP, HPC], F32, tag="dpr")
                den = xp.tile([128, GW, HPC], F32, tag="den")
                rden = xp.tile([128, GW, HPC], F32, tag="rden")
                outh = wp.tile([128, GW, EC], BF16, tag="outh")
                outh_g[g] = outh
                for p in range(NP):
                    nc.vector.tensor_tensor(
                        out=pr[:, :, p, :], in0=t[:, :, p, 0:256],
                        in1=kvt[:, :, 256 * p:256 * (p + 1)], op=OP.mult)
                nc.vector.tensor_add(s01, pr[:, :, 0, :], pr[:, :, 1, :])
                nc.vector.tensor_add(s23, pr[:, :, 2, :], pr[:, :, 3, :])
                nc.vector.tensor_add(s01, s01, pr[:, :, 4, :])
                nc.vector.tensor_tensor(out=numq, in0=s01, in1=s23, op=OP.add)
                # den
                nc.vector.tensor_tensor(
                    out=dpr, in0=qs[:, :, :, 0:4],
                    in1=kvt[:, :, 1280:1300].rearrange("a s (p h) -> a s p h",
                                                       h=HPC),
                    op=OP.mult)
                nc.vector.tensor_reduce(out=den,
                                        in_=dpr.rearrange("a s p h -> a s h p"),
                                        axis=AX, op=OP.add)
                nc.vector.tensor_scalar_add(out=den, in0=den, scalar1=EPS_DEN)
                nc.vector.reciprocal(out=rden, in_=den)
                rdb = xp.tile([128, GW, EC], BF16, tag="rdb")
                nc.scalar.activation(
                    out=rdb.rearrange("a s (h d) -> a s h d", h=HPC),
                    in_=_bcast(rden, DH), func=ACT.Copy)
                nc.vector.tensor_tensor(out=outh, in0=numq, in1=rdb,
                                        op=OP.mult)

            def proj_site(i):
                g, j = divmod(i, GW)
                outh = outh_g[g]
                si = slice(128 * i, 128 * (i + 1))
                tp = ps_t.tile([128, 2, 128], BF16, tag="tp")
                outt = sp.tile([128, 2, 128], BF16, tag="outt")
                for kt in range(2):
                    nc.tensor.transpose(tp[:, kt, :],
                                        outh[:, j, 128 * kt:128 * (kt + 1)],
                                        ident)
                nc.scalar.copy(out=outt, in_=tp)
                outf = sp.tile([128, D], BF16, tag="outf")
                op0 = ps_o.tile([128, 512], F32, tag="op0")
                op1 = ps_o.tile([128, 512], F32, tag="op1")
                for kt in range(2):
                    nc.tensor.matmul(op0, outt[:, kt, :],
                                     woutt[:, kt, 0:512],
                                     start=(kt == 0), stop=(kt == 1))
                    nc.tensor.matmul(op1, outt[:, kt, :],
                                     woutt[:, kt, 512:1024],
                                     start=(kt == 0), stop=(kt == 1))
                nc.vector.tensor_copy(out=outf[:, 0:512], in_=op0)
                nc.scalar.copy(out=outf[:, 512:1024], in_=op1)
                nc.sync.dma_start(out=PART[si, 0:512], in_=outf[:, 0:512])
                nc.sync.dma_start(out=PART[si, 512:1024], in_=outf[:, 512:1024])

            # ---------------- software-pipelined emission ----------------
            qkv_site(0)
            qkv_site(1)
            cheb_group(0)
            qkv_site(2)
            qkv_site(3)
            cumsum_site(0)
            cumsum_site(1)
            cheb_group(1)
            qkv_site(4)
            qkv_site(5)
            numden_group(0)
            cumsum_site(2)
            cumsum_site(3)
            cheb_group(2)
            qkv_site(6)
            qkv_site(7)
            proj_site(0)
            proj_site(1)
            numden_group(1)
            cumsum_site(4)
            cumsum_site(5)
            cheb_group(3)
            proj_site(2)
            proj_site(3)
            numden_group(2)
            cumsum_site(6)
            cumsum_site(7)
            proj_site(4)
            proj_site(5)
            numden_group(3)
            proj_site(6)
            proj_site(7)

    nc.compile()
    return nc


_NC = None


def _get_nc():
    global _NC
    if _NC is None:
        _NC = _build()
    return _NC


def _stage_inputs(x, W_in, W_out):
    import ml_dtypes
    bf = ml_dtypes.bfloat16
    beta = _beta()
    tri = np.tril(np.ones((128, 128), np.float32))
    ltb = np.stack([beta[p] * tri for p in range(1, 6)]).astype(bf)
    lt1 = tri.astype(bf)
    ident = np.eye(128, dtype=bf)
    beta5 = np.tile(np.repeat(beta[1:6], HPC)[None, :], (128, 1)).astype(bf)
    in_maps = []
    for c in range(8):
        b, hb = divmod(c, 4)
        rs = slice(256 * hb, 256 * (hb + 1))
        wq = W_in[0 * D + 256 * hb:0 * D + 256 * (hb + 1)] * INV_SQRT_D
        wk = W_in[1 * D + 256 * hb:1 * D + 256 * (hb + 1)] * INV_SQRT_D
        wv = W_in[2 * D + 256 * hb:2 * D + 256 * (hb + 1)]
        wqkvt = np.ascontiguousarray(
            np.concatenate([wq, wk, wv], axis=0).T).astype(bf)
        xrev = x[b].T.reshape(D, NS, 128)[:, :, ::-1].reshape(D, S)
        in_maps.append({
            "xt": np.ascontiguousarray(xrev).astype(bf),
            "wqkvt": wqkvt,
            "woutt": np.ascontiguousarray(W_out[:, rs].T).astype(bf),
            "ltb": ltb,
            "lt1": lt1,
            "ident": ident,
            "beta5": beta5,
        })
    return in_maps


def _gather(results):
    out = np.zeros((B, S, D), dtype=np.float32)
    for c in range(8):
        part = results[c]["part"].astype(np.float32)
        part = part.reshape(NS, 128, D)[:, ::-1, :].reshape(S, D)
        out[c // 4] += part
    return out


def kernel(x, W_in, W_out):
    from concourse.bass_utils import run_bass_kernel_spmd

    x = np.asarray(x, dtype=np.float32)
    W_in = np.asarray(W_in, dtype=np.float32)
    W_out = np.asarray(W_out, dtype=np.float32)
    nc = _get_nc()
    in_maps = _stage_inputs(x, W_in, W_out)
    res = run_bass_kernel_spmd(nc, in_maps, core_ids=list(range(8)))
    return _gather(res.results)
